# revision 1
# baseline (speedup 1.0000x reference)
"""Multi-head attention Trainium2 kernel (B=8, S=2048, EMB=768, H=4, Dh=192).

Strategy: data-parallel over batch — one batch element per NeuronCore, no
collectives. Inside each core everything runs in "feature-on-partition"
(transposed) layouts so the PE contraction dim always sits on partitions:

  P1: QT[do,s] = WqT.T @ qT   (and KT; V computed in natural [s,do] layout)
  P2: per head h, per q-block of 512:
        scoresT[k,q] = KhT.T @ QhT        (k on partitions -> attV needs no
        E = exp(scoresT/sqrt(Dh))          transpose; softmax denominator
        outT[dh,q]  = Vh.T @ E             comes from a ones column added to
        Z[q]        = ones.T @ E           V, riding the same matmuls)
        outT       *= 1/Z  (broadcast via K=1 PE matmul)
  P3: out[s,e] = OcatT.T @ WoT (+bo via ones row)

All matmuls run as float32r (TF32-like: full speed, ~1.5e-4 rel err).
Host pre-transposes inputs/weights (layout-only, no FLOPs on host).
Biases: bq/bk fused into the ACT psum-evacuation (per-partition bias);
bv/bo folded in via ones-row augmented weights.
"""

import sys

sys.path.insert(0, "/opt/trn_rl_repo")

import numpy as np

import concourse.bass as bass  # noqa: F401  (import keeps bass registered)
import concourse.mybir as mybir
import concourse.tile as tile
from concourse import bacc

B, S, EMB, HEADS = 8, 2048, 768, 4
DH = EMB // HEADS  # 192
NCORES = 8
P = 128
DI_TILES = EMB // P  # 6
S_TILES = S // P  # 16
QBLK = 512
N_QBLK = S // QBLK  # 4
EBLK = 384
SCALE = 1.0 / float(np.sqrt(DH))

F32 = mybir.dt.float32
F32R = mybir.dt.float32r
MMDT = mybir.dt.float16  # matmul operand dtype (psum accumulation is fp32)


def _np_mmdt():
    return np.float16


def _build_nc(reps=1, phases=3):
    nc = bacc.Bacc("TRN2", target_bir_lowering=False, debug=False,
                   num_devices=NCORES)

    xq = nc.declare_dram_parameter("xq", [EMB, S], MMDT, isOutput=False)
    xk = nc.declare_dram_parameter("xk", [EMB, S], MMDT, isOutput=False)
    xv = nc.declare_dram_parameter("xv", [EMB + 1, S], MMDT, isOutput=False)
    wq = nc.declare_dram_parameter("wq", [EMB, EMB], MMDT, isOutput=False)
    wk = nc.declare_dram_parameter("wk", [EMB, EMB], MMDT, isOutput=False)
    wv = nc.declare_dram_parameter("wv", [EMB + 1, EMB], MMDT, isOutput=False)
    wo = nc.declare_dram_parameter("wo", [EMB + 1, EMB], MMDT, isOutput=False)
    bq = nc.declare_dram_parameter("bq", [EMB, 1], F32, isOutput=False)
    bk = nc.declare_dram_parameter("bk", [EMB, 1], F32, isOutput=False)
    onesd = nc.declare_dram_parameter("onesd", [P, S], MMDT, isOutput=False)
    out = nc.declare_dram_parameter("out", [S, EMB], F32, isOutput=True)

    VW = DH + 1  # 193 cols per head in the V tile (192 dh + ones)

    with tile.TileContext(nc) as tc:
        with tc.tile_pool(name="dram", bufs=1, space="DRAM") as dpool, \
             tc.tile_pool(name="res", bufs=1) as res:
            # DRAM scratch: only Q^T / K^T bounce (per head, so phase 2 can
            # start while phase 1 is still finishing)
            qt_h = [dpool.tile([DH, S], MMDT, name=f"qt{h}", tag=f"qt{h}") for h in range(HEADS)]
            kt_h = [dpool.tile([DH, S], MMDT, name=f"kt{h}", tag=f"kt{h}") for h in range(HEADS)]

            # SBUF residents: V (natural layout, per-head 192 cols + ones
            # col) and the concatenated attention output O^T
            v_sb = res.tile([P, S_TILES, HEADS * VW], MMDT, name="v_sb")
            oc_sb = [res.tile([P, S], MMDT, name=f"oc{j}", tag=f"oc{j}")
                     for j in range(DI_TILES)]

            ones_row = res.tile([1, S], MMDT, name="ones_row")
            nc.sync.dma_start(out=ones_row, in_=onesd[0:1, :])
            ones_bcast = res.tile([1, P], MMDT, name="ones_bcast")
            nc.sync.dma_start(out=ones_bcast, in_=onesd[0:1, 0:P])
            # all 4 heads' ones columns in V, one DMA
            nc.sync.dma_start(
                out=v_sb.rearrange("p t (h c) -> p t h c", c=VW)[:, :, :, DH],
                in_=onesd[:, 0:S_TILES * HEADS].rearrange(
                    "p (t h) -> p t h", h=HEADS))

            for rep in range(reps):
                # ---------------- Phase 1a: V projection ----------------
                with tc.tile_pool(name=f"wv1_{rep}", bufs=1) as wvp, \
                     tc.tile_pool(name=f"xv1_{rep}", bufs=3) as xvp, \
                     tc.tile_pool(name=f"psv_{rep}", bufs=2, space="PSUM") as psv:
                    # V first: attV for every head gates on it
                    wv_t = [wvp.tile([P, EMB], MMDT, name=f"wv{i}", tag=f"wv{i}") for i in range(DI_TILES)]
                    wv_last = wvp.tile([1, EMB], MMDT, name="wvl")
                    for i in range(DI_TILES):
                        nc.sync.dma_start(out=wv_t[i], in_=wv[i * P:(i + 1) * P, :])
                    nc.sync.dma_start(out=wv_last, in_=wv[EMB:EMB + 1, :])

                    for sb in range(N_QBLK):
                        vs = []
                        for j in range(DI_TILES):
                            t = xvp.tile([P, QBLK], MMDT, name=f"vs{j}", tag=f"vs{j}")
                            nc.sync.dma_start(
                                out=t, in_=xv[j * P:(j + 1) * P,
                                              sb * QBLK:(sb + 1) * QBLK])
                            vs.append(t)
                        vlast = xvp.tile([1, QBLK], MMDT, name="vsl")
                        nc.sync.dma_start(
                            out=vlast, in_=xv[EMB:EMB + 1, sb * QBLK:(sb + 1) * QBLK])
                        for sti in range(QBLK // P):
                            st = sb * (QBLK // P) + sti
                            pcols = slice(sti * P, (sti + 1) * P)
                            for blk in range(2):  # heads {0,1} then {2,3}
                                ps = psv.tile([P, EBLK], F32, name="vproj", tag="vproj")
                                for di in range(DI_TILES):
                                    nc.tensor.matmul(
                                        ps, vs[di][:, pcols],
                                        wv_t[di][:, blk * EBLK:(blk + 1) * EBLK],
                                        start=(di == 0), stop=False)
                                nc.tensor.matmul(
                                    ps, vlast[:, pcols],
                                    wv_last[:, blk * EBLK:(blk + 1) * EBLK],
                                    start=False, stop=True)
                                # evacuate into v_sb with per-head gap for ones col
                                dst = v_sb[:, st, blk * 2 * VW:(blk * 2 + 2) * VW]
                                dst = dst.rearrange("p (h c) -> p h c", c=VW)[:, :, 0:DH]
                                nc.scalar.copy(
                                    dst, ps.rearrange("p (h c) -> p h c", c=DH))

                # ---------------- Phase 1b: K and Q projections ----------------
                with tc.tile_pool(name=f"w1_{rep}", bufs=1) as wp, \
                     tc.tile_pool(name=f"x1_{rep}", bufs=3) as xp, \
                     tc.tile_pool(name=f"ev1_{rep}", bufs=4) as evp, \
                     tc.tile_pool(name=f"ps1_{rep}", bufs=3, space="PSUM") as psp:
                    wq_t = [wp.tile([P, EMB], MMDT, name=f"wq{i}", tag=f"wq{i}") for i in range(DI_TILES)]
                    wk_t = [wp.tile([P, EMB], MMDT, name=f"wk{i}", tag=f"wk{i}") for i in range(DI_TILES)]
                    bq_t = [wp.tile([P, 1], F32, name=f"bq{i}", tag=f"bq{i}") for i in range(DI_TILES)]
                    bk_t = [wp.tile([P, 1], F32, name=f"bk{i}", tag=f"bk{i}") for i in range(DI_TILES)]
                    for i in range(DI_TILES):
                        nc.sync.dma_start(out=wk_t[i], in_=wk[i * P:(i + 1) * P, :])
                        nc.sync.dma_start(out=bk_t[i], in_=bk[i * P:(i + 1) * P, :])
                    for i in range(DI_TILES):
                        nc.sync.dma_start(out=wq_t[i], in_=wq[i * P:(i + 1) * P, :])
                        nc.sync.dma_start(out=bq_t[i], in_=bq[i * P:(i + 1) * P, :])

                    for (xin, wt, bt, dst_h) in ((xk, wk_t, bk_t, kt_h),
                                                 (xq, wq_t, bq_t, qt_h)):
                        for sb in range(N_QBLK):
                            xs = []
                            for j in range(DI_TILES):
                                t = xp.tile([P, QBLK], MMDT, name=f"xs{j}", tag=f"xs{j}")
                                nc.sync.dma_start(
                                    out=t, in_=xin[j * P:(j + 1) * P,
                                                   sb * QBLK:(sb + 1) * QBLK])
                                xs.append(t)
                            for do in range(DI_TILES):
                                ps = psp.tile([P, QBLK], F32, name="proj", tag="proj")
                                for di in range(DI_TILES):
                                    nc.tensor.matmul(
                                        ps, wt[di][:, do * P:(do + 1) * P],
                                        xs[di][:, :],
                                        start=(di == 0), stop=(di == DI_TILES - 1))
                                ev = evp.tile([P, QBLK], MMDT, name="ev", tag="ev")
                                nc.scalar.activation(
                                    ev, ps, mybir.ActivationFunctionType.Identity,
                                    bias=bt[do], scale=1.0)
                                r0 = do * P
                                while r0 < (do + 1) * P:
                                    h = r0 // DH
                                    r1 = min((do + 1) * P, (h + 1) * DH)
                                    nc.gpsimd.dma_start(
                                        out=dst_h[h][r0 - h * DH:r1 - h * DH,
                                                     sb * QBLK:(sb + 1) * QBLK],
                                        in_=ev[r0 - do * P:r1 - do * P, :])
                                    r0 = r1

                if phases >= 2:
                    # ---------------- Phase 2: attention ----------------
                    with tc.tile_pool(name=f"kh_{rep}", bufs=2) as khp, \
                         tc.tile_pool(name=f"qh_{rep}", bufs=2) as qhp, \
                         tc.tile_pool(name=f"ee_{rep}", bufs=2) as eep, \
                         tc.tile_pool(name=f"zz_{rep}", bufs=3) as zzp, \
                         tc.tile_pool(name=f"pse_{rep}", bufs=3, space="PSUM") as pse, \
                         tc.tile_pool(name=f"pso_{rep}", bufs=2, space="PSUM") as pso, \
                         tc.tile_pool(name=f"psb_{rep}", bufs=1, space="PSUM") as psb:
                        for h in range(HEADS):
                            kh_lo = khp.tile([P, S], MMDT, name="khlo", tag="khlo")
                            nc.sync.dma_start(out=kh_lo, in_=kt_h[h][0:P, :])
                            kh_hi = khp.tile([DH - P, S], MMDT, name="khhi", tag="khhi")
                            nc.sync.dma_start(out=kh_hi, in_=kt_h[h][P:DH, :])
                            for qb in range(N_QBLK):
                                qcols = slice(qb * QBLK, (qb + 1) * QBLK)
                                qh_lo = qhp.tile([P, QBLK], MMDT, name="qhlo", tag="qhlo")
                                nc.sync.dma_start(out=qh_lo, in_=qt_h[h][0:P, qcols])
                                qh_hi = qhp.tile([DH - P, QBLK], MMDT, name="qhhi", tag="qhhi")
                                nc.sync.dma_start(out=qh_hi, in_=qt_h[h][P:DH, qcols])

                                e_all = eep.tile([P, S_TILES, QBLK], MMDT, name="E", tag="E")
                                for kt in range(S_TILES):
                                    kc = slice(kt * P, (kt + 1) * P)
                                    ps_e = pse.tile([P, QBLK], F32, name="pse", tag="pse")
                                    nc.tensor.matmul(ps_e, kh_lo[:, kc], qh_lo[:, :],
                                                     start=True, stop=False)
                                    nc.tensor.matmul(ps_e, kh_hi[:, kc], qh_hi[:, :],
                                                     start=False, stop=True)
                                    nc.scalar.activation(
                                        e_all[:, kt, :], ps_e,
                                        mybir.ActivationFunctionType.Exp,
                                        bias=0.0, scale=SCALE)

                                ps_o1 = pso.tile([P, QBLK], F32, name="o1", tag="o1")
                                ps_o2 = pso.tile([DH + 1 - P, QBLK], F32, name="o2", tag="o2")
                                for kt in range(S_TILES):
                                    nc.tensor.matmul(
                                        ps_o1, v_sb[:, kt, h * VW:h * VW + P],
                                        e_all[:, kt, :],
                                        start=(kt == 0), stop=(kt == S_TILES - 1))
                                    nc.tensor.matmul(
                                        ps_o2, v_sb[:, kt, h * VW + P:(h + 1) * VW],
                                        e_all[:, kt, :],
                                        start=(kt == 0), stop=(kt == S_TILES - 1))
                                rz = zzp.tile([1, QBLK], MMDT, name="rz", tag="rz")
                                with nc.allow_low_precision(
                                        reason="softmax reciprocal, fp32r storage"):
                                    nc.vector.reciprocal(rz, ps_o2[DH - P:DH - P + 1, :])
                                ps_b = psb.tile([P, QBLK], F32, name="psb", tag="psb")
                                nc.tensor.matmul(ps_b, ones_bcast[:, :], rz[:, :],
                                                 start=True, stop=True)
                                bz = zzp.tile([P, QBLK], F32, name="bz", tag="bz")
                                nc.scalar.copy(bz, ps_b)
                                # normalize straight into the resident O^T tiles,
                                # splitting on 128-row SBUF tile boundaries
                                segs = sorted({0, DH, P} |
                                              {j * P - h * DH for j in range(DI_TILES + 1)
                                               if 0 < j * P - h * DH < DH})
                                with nc.allow_low_precision(
                                        reason="softmax normalize, fp32r storage"):
                                    for a, b in zip(segs[:-1], segs[1:]):
                                        r = h * DH + a
                                        j, p0 = divmod(r, P)
                                        src = (ps_o1[a:b, :] if b <= P
                                               else ps_o2[a - P:b - P, :])
                                        nc.vector.tensor_mul(
                                            oc_sb[j][p0:p0 + (b - a), qcols],
                                            src, bz[0:b - a, :])

                if phases >= 3:
                    # ---------------- Phase 3: output projection ----------------
                    with tc.tile_pool(name=f"w3_{rep}", bufs=1) as wp3, \
                         tc.tile_pool(name=f"ev3_{rep}", bufs=4) as evp3, \
                         tc.tile_pool(name=f"ps3_{rep}", bufs=4, space="PSUM") as psp3:
                        wo_t = [wp3.tile([P, EMB], MMDT, name=f"wo{i}", tag=f"wo{i}") for i in range(DI_TILES)]
                        wo_last = wp3.tile([1, EMB], MMDT, name="wol")
                        for i in range(DI_TILES):
                            nc.sync.dma_start(out=wo_t[i], in_=wo[i * P:(i + 1) * P, :])
                        nc.sync.dma_start(out=wo_last, in_=wo[EMB:EMB + 1, :])

                        for st in range(S_TILES):
                            scols = slice(st * P, (st + 1) * P)
                            for eb in range(2):
                                ecols = slice(eb * EBLK, (eb + 1) * EBLK)
                                ps = psp3.tile([P, EBLK], F32, name="fin", tag="fin")
                                for j in range(DI_TILES):
                                    nc.tensor.matmul(ps, oc_sb[j][:, scols],
                                                     wo_t[j][:, ecols],
                                                     start=(j == 0), stop=False)
                                nc.tensor.matmul(ps, ones_row[0:1, scols],
                                                 wo_last[:, ecols],
                                                 start=False, stop=True)
                                fin = evp3.tile([P, EBLK], F32, name="fin_sb", tag="fin_sb")
                                nc.scalar.copy(fin, ps)
                                nc.gpsimd.dma_start(out=out[scols, ecols], in_=fin)

    nc.compile()
    return nc


_CACHE = {}


def _get_runner(reps=1, phases=3):
    """Build nc once and a reusable jitted SPMD callable (no recompiles)."""
    key = f"runner{reps}_{phases}"
    if key in _CACHE:
        return _CACHE[key]

    import jax
    import numpy as _np
    from jax.sharding import Mesh, PartitionSpec
    from jax.experimental.shard_map import shard_map
    from concourse import bass2jax
    from concourse.bass2jax import _bass_exec_p, install_neuronx_cc_hook

    nc = _build_nc(reps, phases)
    install_neuronx_cc_hook()

    partition_name = (nc.partition_id_tensor.name
                      if nc.partition_id_tensor else None)
    in_names, out_names, out_avals, zero_outs = [], [], [], []
    for alloc in nc.m.functions[0].allocations:
        if not isinstance(alloc, mybir.MemoryLocationSet):
            continue
        name = alloc.memorylocations[0].name
        if alloc.kind == "ExternalInput":
            if name != partition_name:
                in_names.append(name)
        elif alloc.kind == "ExternalOutput":
            shape = list(alloc.tensor_shape)
            npdt = mybir.dt.np(alloc.dtype)
            out_avals.append(jax.core.ShapedArray(shape, npdt))
            out_names.append(name)
            zero_outs.append(_np.zeros(shape, npdt))
    n_params = len(in_names)
    n_outs = len(out_names)
    in_names = in_names + out_names
    if partition_name is not None:
        in_names.append(partition_name)

    def _body(*args):
        operands = list(args)
        if partition_name is not None:
            operands.append(bass2jax.partition_id_tensor())
        outs = _bass_exec_p.bind(
            *operands,
            out_avals=tuple(out_avals),
            in_names=tuple(in_names),
            out_names=tuple(out_names),
            lowering_input_output_aliases=(),
            sim_require_finite=True,
            sim_require_nnan=True,
            nc=nc,
        )
        return tuple(outs)

    devices = jax.devices()[:NCORES]
    mesh = Mesh(_np.asarray(devices), ("core",))
    in_specs = (PartitionSpec("core"),) * (n_params + n_outs)
    out_specs = (PartitionSpec("core"),) * n_outs
    sharded = jax.jit(
        shard_map(_body, mesh=mesh, in_specs=in_specs, out_specs=out_specs,
                  check_rep=False),
        keep_unused=True,
    )
    concat_zeros = [
        _np.zeros((NCORES * z.shape[0], *z.shape[1:]), z.dtype)
        for z in zero_outs
    ]

    runner = {
        "nc": nc, "sharded": sharded, "in_names": in_names,
        "n_params": n_params, "out_names": out_names,
        "out_avals": out_avals, "concat_zeros": concat_zeros,
        "mesh": mesh,
    }
    _CACHE[key] = runner
    return runner


def run_spmd(in_maps):
    """Run the compiled SPMD program; in_maps is a list of NCORES dicts."""
    import numpy as _np
    r = _get_runner()
    per_core = [[_np.asarray(m[name]) for name in r["in_names"][:r["n_params"]]]
                for m in in_maps]
    concat_in = [
        _np.concatenate([per_core[c][i] for c in range(NCORES)], axis=0)
        for i in range(r["n_params"])
    ]
    out_arrs = r["sharded"](*concat_in, *r["concat_zeros"])
    return [
        {name: _np.asarray(out_arrs[i]).reshape(NCORES, *r["out_avals"][i].shape)[c]
         for i, name in enumerate(r["out_names"])}
        for c in range(NCORES)
    ]


def _prep_in_maps(q, k, v, Wq, bq, Wk, bk, Wv, bv, Wo, bo):
    mdt = _np_mmdt()
    q = np.asarray(q, dtype=np.float32)
    k = np.asarray(k, dtype=np.float32)
    v = np.asarray(v, dtype=np.float32)
    wqT = np.ascontiguousarray(np.asarray(Wq, np.float32).T).astype(mdt)
    wkT = np.ascontiguousarray(np.asarray(Wk, np.float32).T).astype(mdt)
    wvT = np.ascontiguousarray(
        np.concatenate([np.asarray(Wv, np.float32).T,
                        np.asarray(bv, np.float32)[None, :]],
                       axis=0)).astype(mdt)
    woT = np.ascontiguousarray(
        np.concatenate([np.asarray(Wo, np.float32).T,
                        np.asarray(bo, np.float32)[None, :]],
                       axis=0)).astype(mdt)
    bqc = np.ascontiguousarray(np.asarray(bq, np.float32).reshape(EMB, 1))
    bkc = np.ascontiguousarray(np.asarray(bk, np.float32).reshape(EMB, 1))
    ones = np.ones((P, S), dtype=mdt)
    in_maps = []
    for b in range(NCORES):
        xvT = np.concatenate(
            [v[b].T, np.ones((1, S), np.float32)], axis=0)
        in_maps.append({
            "xq": np.ascontiguousarray(q[b].T).astype(mdt),
            "xk": np.ascontiguousarray(k[b].T).astype(mdt),
            "xv": np.ascontiguousarray(xvT).astype(mdt),
            "wq": wqT, "wk": wkT, "wv": wvT, "wo": woT,
            "bq": bqc, "bk": bkc, "onesd": ones,
        })
    return in_maps


def kernel(q, k, v, Wq, bq, Wk, bk, Wv, bv, Wo, bo):
    in_maps = _prep_in_maps(q, k, v, Wq, bq, Wk, bk, Wv, bv, Wo, bo)
    results = run_spmd(in_maps)
    out = np.stack([results[b]["out"] for b in range(NCORES)], axis=0)
    return out.astype(np.float32)



# revision 3
# speedup vs baseline: 1.6416x; 1.6416x over previous
"""Multi-head attention Trainium2 kernel (B=8, S=2048, EMB=768, H=4, Dh=192).

Strategy: data-parallel over batch — one batch element per NeuronCore, no
collectives. v2: everything SBUF-resident (no DRAM bounce of Q^T/K^T),
attention software-pipelined so the ACT exp stream always has work, output
projection interleaved per q-block.

Layouts (feature-on-partition everywhere except V):
  P1: QT[do,s] = Wq^T.T @ q^T  -> qt_sb (6 resident [128,2048] tiles); same KT.
      V[s,do] natural          -> v_sb  [128, 16, 4*(192+1)] (+ones col per head)
  P2: per block (qb, h), reading head segments straight out of qt/kt tiles
      via base-partition slicing (Dh=192 = 128 + 64 with tile_position):
        scoresT[k,q] = Kh^T.T @ Qh^T   (k on partitions)
        E = exp(scoresT*scale)          (ACT)
        outT[dh,q] = Vh.T @ E ; Z[q] = ones.T @ E  (rides V's ones column)
        outT *= 1/Z (broadcast via K=1 PE matmul; normalize on DVE into oc)
  P3: out[s,e] = Oc^T.T @ Wo^T (+bo via ones row), per q-block right after
      its 4 heads finish.
"""

import sys

sys.path.insert(0, "/opt/trn_rl_repo")

import numpy as np

import concourse.bass as bass  # noqa: F401  (import keeps bass registered)
import concourse.mybir as mybir
import concourse.tile as tile
from concourse import bacc

B, S, EMB, HEADS = 8, 2048, 768, 4
DH = EMB // HEADS  # 192
NCORES = 8
P = 128
DI_TILES = EMB // P  # 6
S_TILES = S // P  # 16
QBLK = 512
N_QBLK = S // QBLK  # 4
EBLK = 384
SCALE = 1.0 / float(np.sqrt(DH))
VW = DH + 1  # 193 cols per head in the V tile (192 dh + ones)

F32 = mybir.dt.float32
MMDT = mybir.dt.float16  # matmul operand dtype (psum accumulation is fp32)


def _np_mmdt():
    return np.float16


# per-head (seg_tile_idx, row0, row1) pairs covering rows h*DH..(h+1)*DH of
# the 6x[128, S] transposed projection tiles
def _head_segs(h):
    segs = []
    r0 = h * DH
    r1 = (h + 1) * DH
    while r0 < r1:
        j, p0 = divmod(r0, P)
        p1 = min(P, p0 + (r1 - r0))
        segs.append((j, p0, p1))
        r0 += p1 - p0
    return segs


def _build_nc(reps=1, phases=3):
    nc = bacc.Bacc("TRN2", target_bir_lowering=False, debug=False,
                   num_devices=NCORES)

    xq = nc.declare_dram_parameter("xq", [EMB, S], MMDT, isOutput=False)
    xk = nc.declare_dram_parameter("xk", [EMB, S], MMDT, isOutput=False)
    xv = nc.declare_dram_parameter("xv", [EMB + 1, S], MMDT, isOutput=False)
    wq = nc.declare_dram_parameter("wq", [EMB, EMB], MMDT, isOutput=False)
    wk = nc.declare_dram_parameter("wk", [EMB, EMB], MMDT, isOutput=False)
    wv = nc.declare_dram_parameter("wv", [EMB + 1, EMB], MMDT, isOutput=False)
    wo = nc.declare_dram_parameter("wo", [EMB + 1, EMB], MMDT, isOutput=False)
    bq = nc.declare_dram_parameter("bq", [EMB, 1], F32, isOutput=False)
    bk = nc.declare_dram_parameter("bk", [EMB, 1], F32, isOutput=False)
    onesd = nc.declare_dram_parameter("onesd", [P, S], MMDT, isOutput=False)
    out = nc.declare_dram_parameter("out", [S, EMB], F32, isOutput=True)

    with tile.TileContext(nc) as tc:
        with tc.tile_pool(name="res", bufs=1) as res, \
             tc.tile_pool(name="psgen", bufs=4, space="PSUM") as psgen:
            # ---- persistent SBUF residents ----
            kt_sb = [res.tile([P, S], MMDT, name=f"kt{j}", tag=f"kt{j}")
                     for j in range(DI_TILES)]
            qt_sb = [res.tile([P, S], MMDT, name=f"qt{j}", tag=f"qt{j}")
                     for j in range(DI_TILES)]
            v_sb = res.tile([P, S_TILES, HEADS * VW], MMDT, name="v_sb")
            oc_sb = [[res.tile([P, QBLK], MMDT, name=f"oc{j}_{qb}",
                               tag=f"oc{j}_{qb}")
                      for qb in range(N_QBLK)] for j in range(DI_TILES)]
            wo_t = [res.tile([P, EMB], MMDT, name=f"wo{i}", tag=f"wo{i}")
                    for i in range(DI_TILES)]
            wo_last = res.tile([1, EMB], MMDT, name="wol")
            ones_row = res.tile([1, S], MMDT, name="ones_row")
            ones_bcast = res.tile([1, P], MMDT, name="ones_bcast")

            nc.sync.dma_start(out=ones_row, in_=onesd[0:1, :])
            nc.sync.dma_start(out=ones_bcast, in_=onesd[0:1, 0:P])
            # all 4 heads' ones columns in V, one DMA
            nc.sync.dma_start(
                out=v_sb.rearrange("p t (h c) -> p t h c", c=VW)[:, :, :, DH],
                in_=onesd[:, 0:S_TILES * HEADS].rearrange(
                    "p (t h) -> p t h", h=HEADS))
            for i in range(DI_TILES):
                nc.sync.dma_start(out=wo_t[i], in_=wo[i * P:(i + 1) * P, :])
            nc.sync.dma_start(out=wo_last, in_=wo[EMB:EMB + 1, :])

            for rep in range(reps):
                # ============ Phase 1: projections (K, V, Q) ============
                with tc.tile_pool(name=f"w1_{rep}", bufs=1) as wp, \
                     tc.tile_pool(name=f"x1_{rep}", bufs=2) as xp:
                    wk_t = [wp.tile([P, EMB], MMDT, name=f"wk{i}", tag=f"wk{i}")
                            for i in range(DI_TILES)]
                    wq_t = [wp.tile([P, EMB], MMDT, name=f"wq{i}", tag=f"wq{i}")
                            for i in range(DI_TILES)]
                    wv_t = [wp.tile([P, EMB], MMDT, name=f"wv{i}", tag=f"wv{i}")
                            for i in range(DI_TILES)]
                    wv_last = wp.tile([1, EMB], MMDT, name="wvl")
                    bq_t = [wp.tile([P, 1], F32, name=f"bq{i}", tag=f"bq{i}")
                            for i in range(DI_TILES)]
                    bk_t = [wp.tile([P, 1], F32, name=f"bk{i}", tag=f"bk{i}")
                            for i in range(DI_TILES)]
                    for i in range(DI_TILES):
                        nc.sync.dma_start(out=wk_t[i], in_=wk[i * P:(i + 1) * P, :])
                        nc.sync.dma_start(out=bk_t[i], in_=bk[i * P:(i + 1) * P, :])
                    for i in range(DI_TILES):
                        nc.sync.dma_start(out=wv_t[i], in_=wv[i * P:(i + 1) * P, :])
                    nc.sync.dma_start(out=wv_last, in_=wv[EMB:EMB + 1, :])
                    for i in range(DI_TILES):
                        nc.sync.dma_start(out=wq_t[i], in_=wq[i * P:(i + 1) * P, :])
                        nc.sync.dma_start(out=bq_t[i], in_=bq[i * P:(i + 1) * P, :])

                    # K then Q: transposed projections into resident tiles
                    for (xin, wt, bt, dst) in ((xk, wk_t, bk_t, kt_sb),
                                               (xq, wq_t, bq_t, qt_sb)):
                        for sb in range(N_QBLK):
                            scols = slice(sb * QBLK, (sb + 1) * QBLK)
                            xs = []
                            for j in range(DI_TILES):
                                t = xp.tile([P, QBLK], MMDT, name=f"xs{j}",
                                            tag=f"xs{j}")
                                nc.sync.dma_start(out=t, in_=xin[j * P:(j + 1) * P,
                                                                scols])
                                xs.append(t)
                            for do in range(DI_TILES):
                                ps = psgen.tile([P, QBLK], F32, name="gen",
                                                tag="gen")
                                for di in range(DI_TILES):
                                    nc.tensor.matmul(
                                        ps, wt[di][:, do * P:(do + 1) * P],
                                        xs[di][:, :],
                                        start=(di == 0), stop=(di == DI_TILES - 1))
                                with nc.allow_low_precision(
                                        reason="fp16 storage of projections"):
                                    nc.vector.tensor_scalar_add(
                                        dst[do][:, scols], ps, bt[do])

                        if xin is xk:
                            # V projection (natural layout, per-head ones col)
                            for sb in range(N_QBLK):
                                scols = slice(sb * QBLK, (sb + 1) * QBLK)
                                vs = []
                                for j in range(DI_TILES):
                                    t = xp.tile([P, QBLK], MMDT, name=f"vs{j}",
                                                tag=f"vs{j}")
                                    nc.sync.dma_start(
                                        out=t, in_=xv[j * P:(j + 1) * P, scols])
                                    vs.append(t)
                                vlast = xp.tile([1, QBLK], MMDT, name="vsl",
                                                tag="vsl")
                                nc.sync.dma_start(
                                    out=vlast, in_=xv[EMB:EMB + 1, scols])
                                for sti in range(QBLK // P):
                                    st = sb * (QBLK // P) + sti
                                    pcols = slice(sti * P, (sti + 1) * P)
                                    for blk in range(2):  # heads {0,1}, {2,3}
                                        ps = psgen.tile([P, QBLK], F32,
                                                        name="gen", tag="gen")
                                        for di in range(DI_TILES):
                                            nc.tensor.matmul(
                                                ps[:, 0:EBLK], vs[di][:, pcols],
                                                wv_t[di][:, blk * EBLK:(blk + 1) * EBLK],
                                                start=(di == 0), stop=False)
                                        nc.tensor.matmul(
                                            ps[:, 0:EBLK], vlast[:, pcols],
                                            wv_last[:, blk * EBLK:(blk + 1) * EBLK],
                                            start=False, stop=True)
                                        dst = v_sb[:, st,
                                                   blk * 2 * VW:(blk * 2 + 2) * VW]
                                        dst = dst.rearrange(
                                            "p (h c) -> p h c", c=VW)[:, :, 0:DH]
                                        with nc.allow_low_precision(
                                                reason="fp16 storage of V"):
                                            nc.vector.tensor_copy(
                                                dst,
                                                ps[:, 0:EBLK].rearrange(
                                                    "p (h c) -> p h c", c=DH))

                if phases < 2:
                    continue

                # ============ Phase 2+3: pipelined attention ============
                with tc.tile_pool(name=f"ee_{rep}", bufs=2) as eep, \
                     tc.tile_pool(name=f"zz_{rep}", bufs=2) as zzp, \
                     tc.tile_pool(name=f"fe_{rep}", bufs=2) as fep, \
                     tc.tile_pool(name=f"pso1_{rep}", bufs=2, space="PSUM") as pso1, \
                     tc.tile_pool(name=f"pso2_{rep}", bufs=2, space="PSUM") as pso2:

                    blocks = [(qb, h) for qb in range(N_QBLK)
                              for h in range(HEADS)]
                    nb = len(blocks)
                    # live state per in-flight block
                    st_e = {}   # block idx -> e_all tile
                    st_o = {}   # block idx -> (ps_o1, ps_o2)
                    st_rz = {}  # block idx -> rz tile

                    def emit_scores(i):
                        qb, h = blocks[i]
                        qcols = slice(qb * QBLK, (qb + 1) * QBLK)
                        segs = _head_segs(h)
                        e_all = eep.tile([P, S_TILES, QBLK], MMDT,
                                         name="E", tag="E")
                        st_e[i] = e_all
                        for kt in range(S_TILES):
                            kc = slice(kt * P, (kt + 1) * P)
                            ps_e = psgen.tile([P, QBLK], F32, name="gen",
                                              tag="gen")
                            for si, (j, p0, p1) in enumerate(segs):
                                nc.tensor.matmul(
                                    ps_e, kt_sb[j][p0:p1, kc],
                                    qt_sb[j][p0:p1, qcols],
                                    start=(si == 0), stop=(si == len(segs) - 1))
                            nc.scalar.activation(
                                e_all[:, kt, :], ps_e,
                                mybir.ActivationFunctionType.Exp,
                                bias=0.0, scale=SCALE)

                    def emit_attv(i):
                        qb, h = blocks[i]
                        e_all = st_e[i]
                        ps_o1 = pso1.tile([P, QBLK], F32, name="o1", tag="o1")
                        ps_o2 = pso2.tile([DH + 1 - P, QBLK], F32, name="o2",
                                          tag="o2")
                        st_o[i] = (ps_o1, ps_o2)
                        for kt in range(S_TILES):
                            nc.tensor.matmul(
                                ps_o1, v_sb[:, kt, h * VW:h * VW + P],
                                e_all[:, kt, :],
                                start=(kt == 0), stop=(kt == S_TILES - 1))
                            nc.tensor.matmul(
                                ps_o2, v_sb[:, kt, h * VW + P:(h + 1) * VW],
                                e_all[:, kt, :],
                                start=(kt == 0), stop=(kt == S_TILES - 1))
                        rz = zzp.tile([1, QBLK], MMDT, name="rz", tag="rz")
                        st_rz[i] = rz
                        with nc.allow_low_precision(
                                reason="softmax reciprocal, fp16 storage"):
                            nc.vector.reciprocal(rz, ps_o2[DH - P:DH - P + 1, :])

                    def emit_norm(i):
                        qb, h = blocks[i]
                        ps_o1, ps_o2 = st_o.pop(i)
                        rz = st_rz.pop(i)
                        del st_e[i]
                        ps_b = psgen.tile([P, QBLK], F32, name="gen", tag="gen")
                        nc.tensor.matmul(ps_b, ones_bcast[:, :], rz[:, :],
                                         start=True, stop=True)
                        bz = zzp.tile([P, QBLK], F32, name="bz", tag="bz")
                        nc.vector.tensor_copy(bz, ps_b)
                        segs = sorted({0, DH, P} |
                                      {j * P - h * DH for j in range(DI_TILES + 1)
                                       if 0 < j * P - h * DH < DH})
                        with nc.allow_low_precision(
                                reason="softmax normalize, fp16 storage"):
                            for a, b in zip(segs[:-1], segs[1:]):
                                r = h * DH + a
                                j, p0 = divmod(r, P)
                                src = (ps_o1[a:b, :] if b <= P
                                       else ps_o2[a - P:b - P, :])
                                nc.vector.tensor_mul(
                                    oc_sb[j][qb][p0:p0 + (b - a), :],
                                    src, bz[0:b - a, :])

                    def emit_phase3(qb):
                        if phases < 3:
                            return
                        for sti in range(QBLK // P):
                            st = qb * (QBLK // P) + sti
                            scols = slice(st * P, (st + 1) * P)
                            pcols = slice(sti * P, (sti + 1) * P)
                            for eb in range(2):
                                ecols = slice(eb * EBLK, (eb + 1) * EBLK)
                                ps = psgen.tile([P, QBLK], F32, name="gen",
                                                tag="gen")
                                for j in range(DI_TILES):
                                    nc.tensor.matmul(
                                        ps[:, 0:EBLK], oc_sb[j][qb][:, pcols],
                                        wo_t[j][:, ecols],
                                        start=(j == 0), stop=False)
                                nc.tensor.matmul(
                                    ps[:, 0:EBLK], ones_row[0:1, scols],
                                    wo_last[:, ecols],
                                    start=False, stop=True)
                                fin = fep.tile([P, EBLK], F32, name="fin",
                                               tag="fin")
                                nc.vector.tensor_copy(fin, ps[:, 0:EBLK])
                                nc.gpsimd.dma_start(out=out[scols, ecols],
                                                    in_=fin)

                    # software pipeline: scores(i+1) ahead of attV(i);
                    # norm(i) one step behind; phase3 one step behind its
                    # norms so PE never waits on the DVE normalize chain
                    pending_p3 = []
                    emit_scores(0)
                    for i in range(nb):
                        if i + 1 < nb:
                            emit_scores(i + 1)
                        for qb in pending_p3:
                            emit_phase3(qb)
                        pending_p3 = []
                        if i - 1 >= 0:
                            emit_norm(i - 1)
                            qb_prev, h_prev = blocks[i - 1]
                            if h_prev == HEADS - 1:
                                pending_p3.append(qb_prev)
                        emit_attv(i)
                    for qb in pending_p3:
                        emit_phase3(qb)
                    emit_norm(nb - 1)
                    emit_phase3(N_QBLK - 1)

    nc.compile()
    return nc


_CACHE = {}


def _get_runner(reps=1, phases=3):
    """Build nc once and a reusable jitted SPMD callable (no recompiles)."""
    key = f"runner{reps}_{phases}"
    if key in _CACHE:
        return _CACHE[key]

    import jax
    import numpy as _np
    from jax.sharding import Mesh, PartitionSpec
    from jax.experimental.shard_map import shard_map
    from concourse import bass2jax
    from concourse.bass2jax import _bass_exec_p, install_neuronx_cc_hook

    nc = _build_nc(reps, phases)
    install_neuronx_cc_hook()

    partition_name = (nc.partition_id_tensor.name
                      if nc.partition_id_tensor else None)
    in_names, out_names, out_avals, zero_outs = [], [], [], []
    for alloc in nc.m.functions[0].allocations:
        if not isinstance(alloc, mybir.MemoryLocationSet):
            continue
        name = alloc.memorylocations[0].name
        if alloc.kind == "ExternalInput":
            if name != partition_name:
                in_names.append(name)
        elif alloc.kind == "ExternalOutput":
            shape = list(alloc.tensor_shape)
            npdt = mybir.dt.np(alloc.dtype)
            out_avals.append(jax.core.ShapedArray(shape, npdt))
            out_names.append(name)
            zero_outs.append(_np.zeros(shape, npdt))
    n_params = len(in_names)
    n_outs = len(out_names)
    in_names = in_names + out_names
    if partition_name is not None:
        in_names.append(partition_name)

    def _body(*args):
        operands = list(args)
        if partition_name is not None:
            operands.append(bass2jax.partition_id_tensor())
        outs = _bass_exec_p.bind(
            *operands,
            out_avals=tuple(out_avals),
            in_names=tuple(in_names),
            out_names=tuple(out_names),
            lowering_input_output_aliases=(),
            sim_require_finite=True,
            sim_require_nnan=True,
            nc=nc,
        )
        return tuple(outs)

    devices = jax.devices()[:NCORES]
    mesh = Mesh(_np.asarray(devices), ("core",))
    in_specs = (PartitionSpec("core"),) * (n_params + n_outs)
    out_specs = (PartitionSpec("core"),) * n_outs
    sharded = jax.jit(
        shard_map(_body, mesh=mesh, in_specs=in_specs, out_specs=out_specs,
                  check_rep=False),
        keep_unused=True,
    )
    concat_zeros = [
        _np.zeros((NCORES * z.shape[0], *z.shape[1:]), z.dtype)
        for z in zero_outs
    ]

    runner = {
        "nc": nc, "sharded": sharded, "in_names": in_names,
        "n_params": n_params, "out_names": out_names,
        "out_avals": out_avals, "concat_zeros": concat_zeros,
        "mesh": mesh,
    }
    _CACHE[key] = runner
    return runner


def run_spmd(in_maps):
    """Run the compiled SPMD program; in_maps is a list of NCORES dicts."""
    import numpy as _np
    r = _get_runner()
    per_core = [[_np.asarray(m[name]) for name in r["in_names"][:r["n_params"]]]
                for m in in_maps]
    concat_in = [
        _np.concatenate([per_core[c][i] for c in range(NCORES)], axis=0)
        for i in range(r["n_params"])
    ]
    out_arrs = r["sharded"](*concat_in, *r["concat_zeros"])
    return [
        {name: _np.asarray(out_arrs[i]).reshape(NCORES, *r["out_avals"][i].shape)[c]
         for i, name in enumerate(r["out_names"])}
        for c in range(NCORES)
    ]


def _prep_in_maps(q, k, v, Wq, bq, Wk, bk, Wv, bv, Wo, bo):
    mdt = _np_mmdt()
    q = np.asarray(q, dtype=np.float32)
    k = np.asarray(k, dtype=np.float32)
    v = np.asarray(v, dtype=np.float32)
    wqT = np.ascontiguousarray(np.asarray(Wq, np.float32).T).astype(mdt)
    wkT = np.ascontiguousarray(np.asarray(Wk, np.float32).T).astype(mdt)
    wvT = np.ascontiguousarray(
        np.concatenate([np.asarray(Wv, np.float32).T,
                        np.asarray(bv, np.float32)[None, :]],
                       axis=0)).astype(mdt)
    woT = np.ascontiguousarray(
        np.concatenate([np.asarray(Wo, np.float32).T,
                        np.asarray(bo, np.float32)[None, :]],
                       axis=0)).astype(mdt)
    bqc = np.ascontiguousarray(np.asarray(bq, np.float32).reshape(EMB, 1))
    bkc = np.ascontiguousarray(np.asarray(bk, np.float32).reshape(EMB, 1))
    ones = np.ones((P, S), dtype=mdt)
    in_maps = []
    for b in range(NCORES):
        xvT = np.concatenate(
            [v[b].T, np.ones((1, S), np.float32)], axis=0)
        in_maps.append({
            "xq": np.ascontiguousarray(q[b].T).astype(mdt),
            "xk": np.ascontiguousarray(k[b].T).astype(mdt),
            "xv": np.ascontiguousarray(xvT).astype(mdt),
            "wq": wqT, "wk": wkT, "wv": wvT, "wo": woT,
            "bq": bqc, "bk": bkc, "onesd": ones,
        })
    return in_maps


def kernel(q, k, v, Wq, bq, Wk, bk, Wv, bv, Wo, bo):
    in_maps = _prep_in_maps(q, k, v, Wq, bq, Wk, bk, Wv, bv, Wo, bo)
    results = run_spmd(in_maps)
    out = np.stack([results[b]["out"] for b in range(NCORES)], axis=0)
    return out.astype(np.float32)
